# revision 17
# baseline (speedup 1.0000x reference)
"""Bahdanau attention kernel for Trainium2 (Bass/Tile), data-parallel over batch.

Full computation:
    pq    = query[0] @ Wq.T                     # [B, Q]
    e     = einsum('bsq,q->bs', tanh(pq[:,None,:] + pm), We)
    e     = where(mask==0, -1000, e)
    attn  = softmax(e, axis=-1)                 # [B, 1, S]

Strategy:
  * Batch B=64 sharded 8 ways (8 batches/core); Wq, We replicated.
  * Host-side data prep (layout only, no model math): pm is transposed to
    [b, p, qc*cap+j] fp16 so the contraction dim q lands on SBUF partitions
    with one contiguous descriptor per partition and no on-device transposes
    (tolerance is 2e-2; fp16 keeps rel err ~1e-3).  Masked s positions
    contribute exactly 0 to the softmax (exp(-1000-max) == 0 in fp32), so the
    host packs only the unmasked s positions per row (capacity = max count
    rounded up to 64); padded lanes carry weight 0.  This halves both HBM
    traffic and tanh work.
  * Device per batch: DVE adds the projected-query bias per 128-wide q chunk
    (tensor_scalar, 4x perf mode), ACT runs tanh over slabs (quarter slabs
    for batch 0, halves for 1/2/7, full [128, 4*cap] otherwise — sized so
    the ACT train starts as early as DMA allows and stays gap-free), PE
    contracts with a sliding-window We matrix so batch b's energies land on
    PSUM partition row b, accumulating all batches into per-512-chunk PSUM
    tiles (separate tiles give the tail precise per-chunk dependencies).
  * DMA choreography: Wq streams as four per-dc slices on the scalar-engine
    HWDGE ring, pm batches on the sync ring — the two rings drain in
    parallel, so pq chunk dc is ready just in time for tanh quarter qc=dc.
  * Tail: the final batch's matmuls run chunk-major; per-chunk exp (ACT) ->
    fused pad-mask multiply + row-sum (scalar_tensor_tensor accum_out) ->
    reciprocal -> scale -> fp16 DMA out on the idle sync queue; host scatters
    to the full [B, 1, S] fp32 output (masked positions exactly 0).
"""

import sys

if "/opt/trn_rl_repo" not in sys.path:
    sys.path.insert(0, "/opt/trn_rl_repo")

from contextlib import ExitStack

import numpy as np

import concourse.tile as tile
from concourse import bacc, mybir
from concourse.bass_utils import run_bass_kernel_spmd

N_CORES = 8
B, S, Q = 64, 2048, 512
BL = B // N_CORES          # local batches per core
QC = Q // 128              # 128-wide q chunks
WP = 2 * BL - 1            # sliding-window width per q chunk

F32 = mybir.dt.float32
F16 = mybir.dt.float16

_CACHE = {}


def _chunks(cap):
    """Split [0, cap) into <=512-wide pieces aligned to 512 (PSUM banks)."""
    out = []
    c0 = 0
    while c0 < cap:
        out.append((c0, min(c0 + 512, cap)))
        c0 += 512
    return out


def _build(cap):
    nc = bacc.Bacc(
        "TRN2",
        target_bir_lowering=False,
        debug=False,
        enable_asserts=False,
        num_devices=N_CORES,
    )
    pmt_d = nc.dram_tensor("pmt", [BL, 128, QC * cap], F16, kind="ExternalInput").ap()
    # wqa: [qt | wewin | wq dc-major]
    #   qt[p, qc*BL+b] = query[b, qc*128+p]
    #   wq[p, HD + dc*Q + qc*128 + u] = Wq[dc*128 + u, qc*128 + p]
    HD = QC * BL + QC * WP
    wqa_d = nc.dram_tensor("wqa", [128, HD + QC * Q], F16, kind="ExternalInput").ap()
    wmask_d = nc.dram_tensor("wmask", [BL, cap], F16, kind="ExternalInput").ap()
    attn_d = nc.dram_tensor("attn", [BL, cap], F16, kind="ExternalOutput").ap()

    tanh = mybir.ActivationFunctionType.Tanh
    exp = mybir.ActivationFunctionType.Exp
    chunks = _chunks(cap)
    NCH = len(chunks)
    # (b, qc) chunks whose tanh runs as a degree-9 polynomial on the vector
    # engine instead of ACT (balances the two engines); must be qc == QC-1 so
    # the ACT slab for that batch stays contiguous.
    POLY = set()
    PL = 3.2
    PC = (0.96926182, -0.24074044, 0.04296094, -0.00390547, 1.3627379e-04)

    with tile.TileContext(nc) as tc, ExitStack() as ctx:
        setup = ctx.enter_context(tc.tile_pool(name="setup", bufs=1))
        pmp = ctx.enter_context(tc.tile_pool(name="pmp", bufs=4))
        pmh = ctx.enter_context(tc.tile_pool(name="pmh", bufs=4))
        xap = ctx.enter_context(tc.tile_pool(name="xap", bufs=2))
        thp = ctx.enter_context(tc.tile_pool(name="thp", bufs=2))
        ppp = ctx.enter_context(tc.tile_pool(name="ppp", bufs=2))
        pqp = ctx.enter_context(tc.tile_pool(name="pqp", bufs=2, space="PSUM"))
        ep = ctx.enter_context(tc.tile_pool(name="ep", bufs=1, space="PSUM"))
        outp = ctx.enter_context(tc.tile_pool(name="outp", bufs=1))

        # ---- setup DMAs (scalar HWDGE ring, parallel to pm on sync ring) --
        # four slices: [qt|wewin|wq dc0], then wq dc1..dc3; pq group dc needs
        # only the first dc+1 slices, so the first tanh quarter starts early.
        wqa = setup.tile([128, HD + QC * Q], F16)
        nc.scalar.dma_start(wqa[:, : HD + Q], wqa_d[:, : HD + Q])
        for dc in range(1, QC):
            nc.scalar.dma_start(
                wqa[:, HD + dc * Q : HD + (dc + 1) * Q],
                wqa_d[:, HD + dc * Q : HD + (dc + 1) * Q],
            )
        qt_t = wqa[:, : QC * BL]
        wewin = wqa[:, QC * BL : HD]
        wq_t = wqa[:, HD:]
        wmask_t = setup.tile([BL, cap], F16)
        nc.scalar.dma_start(wmask_t[:], wmask_d[:])

        # ---- pm DMAs (sync ring): b0 quarters, b1 halves, b2..b7 full -----
        pm_half = {}
        pm_full = {}
        for h in range(QC):
            t = pmh.tile([128, cap], F16, tag="pmq", name=f"pm_0_{h}")
            nc.sync.dma_start(t[:], pmt_d[0, :, h * cap : (h + 1) * cap])
            pm_half[(0, h)] = t
        for b in (1, 2):
            for h in range(2):
                t = pmh.tile([128, 2 * cap], F16, tag="pmh", name=f"pm_{b}_{h}")
                nc.sync.dma_start(
                    t[:], pmt_d[b, :, h * 2 * cap : (h + 1) * 2 * cap]
                )
                pm_half[(b, h)] = t
        for b in range(3, BL):
            t = pmp.tile([128, QC * cap], F16, tag="pm", name=f"pm_{b}")
            nc.sync.dma_start(t[:], pmt_d[b])
            pm_full[b] = t

        # ---- main loop ----------------------------------------------------
        e_ps = [
            ep.tile([BL, c1 - c0], F32, tag=f"e{ci}", name=f"eps_{ci}")
            for ci, (c0, c1) in enumerate(chunks)
        ]

        def pm_chunk(b, qc):
            if b == 0:
                return pm_half[(0, qc)][:]
            if b in (1, 2):
                return pm_half[(b, qc // 2)][:, (qc % 2) * cap : (qc % 2 + 1) * cap]
            return pm_full[b][:, qc * cap : (qc + 1) * cap]

        # pq: pqT[p, dc*BL + b] = sum_q query[b,q] Wq[dc*128+p, q]
        # emitted interleaved with b0's adds so the DVE queue never puts a
        # late-gated copy ahead of an early add.
        pqT = setup.tile([128, QC * BL], F32)

        def pq_group(dc):
            acc = pqp.tile([128, BL], F32, tag="pq", name=f"pq_{dc}")
            for qc in range(QC):
                nc.tensor.matmul(
                    acc[:],
                    wq_t[:, dc * Q + qc * 128 : dc * Q + (qc + 1) * 128],
                    qt_t[:, qc * BL : (qc + 1) * BL],
                    start=(qc == 0),
                    stop=(qc == QC - 1),
                )
            nc.vector.tensor_copy(pqT[:, dc * BL : (dc + 1) * BL], acc[:])

        # tanh slab split per batch: b0 quarters, b1/b2 halves, b7 half+quarters
        def tanh_parts(b):
            if b == 0:
                return [(qc, qc + 1) for qc in range(QC)]
            if b in (1, 2):
                return [(0, 2), (2, 4)]
            if b == BL - 1:
                return [(0, 2), (2, 3), (3, 4)]
            if (b, QC - 1) in POLY:
                return [(0, QC - 1)]
            return [(0, QC)]

        def emit_add(b, qc):
            nc.vector.tensor_scalar(
                xa_t[b][:, qc * cap : (qc + 1) * cap],
                pm_chunk(b, qc),
                pqT[:, qc * BL + b : qc * BL + b + 1],
                None,
                op0=mybir.AluOpType.add,
            )

        def emit_poly(b, qc):
            """th[:, qc*cap:...] = deg-9 poly tanh of (pm + pq) on DVE."""
            pq_col = pqT[:, qc * BL + b : qc * BL + b + 1]
            xc = ppp.tile([128, cap], F16, tag="xc", name=f"xc_{b}_{qc}")
            t2 = ppp.tile([128, cap], F16, tag="t2", name=f"t2_{b}_{qc}")
            y1 = ppp.tile([128, cap], F16, tag="y1", name=f"y1_{b}_{qc}")
            y2 = ppp.tile([128, cap], F16, tag="y2", name=f"y2_{b}_{qc}")
            A = mybir.AluOpType
            nc.vector.tensor_scalar(xc[:], pm_chunk(b, qc), pq_col, PL,
                                    op0=A.add, op1=A.min)
            nc.vector.tensor_scalar(xc[:], xc[:], -PL, None, op0=A.max)
            nc.vector.tensor_tensor(t2[:], xc[:], xc[:], A.mult)
            nc.vector.tensor_scalar(y1[:], t2[:], PC[4], PC[3],
                                    op0=A.mult, op1=A.add)
            nc.vector.tensor_tensor(y2[:], y1[:], t2[:], A.mult)
            nc.vector.tensor_scalar(y2[:], y2[:], PC[2], None, op0=A.add)
            nc.vector.tensor_tensor(y1[:], y2[:], t2[:], A.mult)
            nc.vector.tensor_scalar(y1[:], y1[:], PC[1], None, op0=A.add)
            nc.vector.tensor_tensor(y2[:], y1[:], t2[:], A.mult)
            nc.vector.tensor_scalar(y2[:], y2[:], PC[0], None, op0=A.add)
            nc.vector.tensor_tensor(
                th_t[b][:, qc * cap : (qc + 1) * cap], y2[:], xc[:], A.mult
            )
            # the chunk's matmuls, deferred here so the writer precedes them
            for c0, c1 in chunks:
                nc.tensor.matmul(
                    e_ps[chunks.index((c0, c1))][:, : c1 - c0],
                    wewin[:, qc * WP + BL - 1 - b : qc * WP + 2 * BL - 1 - b],
                    th_t[b][:, qc * cap + c0 : qc * cap + c1],
                    start=False,
                    stop=False,
                )

        xa_t, th_t = {}, {}
        for b in range(BL):
            xa_t[b] = xap.tile([128, QC * cap], F16, tag="xa", name=f"xa_{b}")
            th_t[b] = thp.tile([128, QC * cap], F16, tag="th", name=f"th_{b}")

        for b in range(BL):
            if b == 0:
                for qc in range(QC):
                    pq_group(qc)
                    emit_add(0, qc)
            else:
                for qc in range(QC):
                    if (b, qc) not in POLY:
                        emit_add(b, qc)
                # deferred poly for an earlier batch: DVE reaches it after
                # this batch's adds, while ACT still works on earlier slabs
                for bp, qp in sorted(POLY):
                    if bp == b - 1:
                        emit_poly(bp, qp)
            th = th_t[b]
            xa = xa_t[b]
            for q0, q1 in tanh_parts(b):
                nc.scalar.activation(
                    th[:, q0 * cap : q1 * cap], xa[:, q0 * cap : q1 * cap], tanh
                )
            # energies: window matmul puts We . th into PSUM row b only
            last = b == BL - 1
            order = (
                [(qc, c) for c in range(NCH) for qc in range(QC)]
                if last
                else [(qc, c) for qc in range(QC) for c in range(NCH)]
            )
            for qc, ci in order:
                if (b, qc) in POLY:
                    continue  # emitted after the deferred poly writes th
                c0, c1 = chunks[ci]
                nc.tensor.matmul(
                    e_ps[ci][:, : c1 - c0],
                    wewin[:, qc * WP + BL - 1 - b : qc * WP + 2 * BL - 1 - b],
                    th[:, qc * cap + c0 : qc * cap + c1],
                    start=(b == 0 and qc == 0),
                    stop=(last and qc == QC - 1),
                )

        # ---- softmax tail (chunked: overlaps the final batch matmuls) ----
        p_e = outp.tile([BL, cap], F16)
        p_m = outp.tile([BL, cap], F16)
        zp = outp.tile([BL, NCH], F32)
        for ci, (c0, c1) in enumerate(chunks):
            nc.scalar.activation(p_e[:, c0:c1], e_ps[ci][:, : c1 - c0], exp)
            nc.vector.scalar_tensor_tensor(
                p_m[:, c0:c1], p_e[:, c0:c1], 1.0, wmask_t[:, c0:c1],
                op0=mybir.AluOpType.mult, op1=mybir.AluOpType.mult,
                accum_out=zp[:, ci : ci + 1],
            )
        z = outp.tile([BL, 1], F32)
        nc.vector.tensor_reduce(
            z[:], zp[:], axis=mybir.AxisListType.X, op=mybir.AluOpType.add
        )
        zr = outp.tile([BL, 1], F32)
        nc.vector.reciprocal(zr[:], z[:])
        a_t = outp.tile([BL, cap], F16)
        nc.vector.tensor_scalar(
            a_t[:], p_m[:], zr[:], None, op0=mybir.AluOpType.mult
        )
        nc.sync.dma_start(attn_d[:], a_t[:])

    nc.compile()
    return nc


def _get_nc(cap):
    if cap not in _CACHE:
        _CACHE[cap] = _build(cap)
    return _CACHE[cap]


def _prep(query, projected_memory, mask, Wq, We):
    query = np.asarray(query, dtype=np.float32)
    pm = np.asarray(projected_memory, dtype=np.float32)
    mask = np.asarray(mask)
    wq = np.asarray(Wq, dtype=np.float32)
    we = np.asarray(We, dtype=np.float32)

    nz = mask != 0
    counts = nz.sum(axis=1).astype(np.int64)
    maxc = int(counts.max()) if counts.size else 0
    cap = min(S, max(128, -(-maxc // 64) * 64))

    idxs = [np.nonzero(nz[b])[0] for b in range(B)]

    # wq in dc-major [128, QC*Q]: wql[p, dc*Q + qc*128 + u] = Wq[dc*128+u, qc*128+p]
    wql = np.ascontiguousarray(
        wq.astype(np.float16).reshape(QC, 128, QC, 128).transpose(3, 0, 2, 1)
    ).reshape(128, QC * Q)
    qt = query[0].T.astype(np.float16)  # [Q, B]
    wewin = np.zeros((128, QC * WP), dtype=np.float16)
    for qc in range(QC):
        wewin[:, qc * WP + BL - 1] = we[qc * 128 : (qc + 1) * 128]

    in_maps = []
    for i in range(N_CORES):
        lo = i * BL
        # qtl[p, qc*BL + b] = query[lo+b, qc*128+p]
        qtl = np.ascontiguousarray(
            qt[:, lo : lo + BL].reshape(QC, 128, BL).transpose(1, 0, 2)
        ).reshape(128, QC * BL)
        wqa = np.concatenate([qtl, wewin, wql], axis=1)
        pmt = np.zeros((BL, 128, QC, cap), dtype=np.float16)
        wmask = np.zeros((BL, cap), dtype=np.float16)
        for bl in range(BL):
            g = lo + bl
            cnt = len(idxs[g])
            if cnt:
                # [cnt, 512] -> [512, cnt] -> [4, 128, cnt] -> [128, 4, cnt]
                r = pm[g][idxs[g], :].astype(np.float16).T
                pmt[bl, :, :, :cnt] = r.reshape(QC, 128, cnt).transpose(1, 0, 2)
                wmask[bl, :cnt] = 1.0
        in_maps.append(
            {"pmt": np.ascontiguousarray(pmt.reshape(BL, 128, QC * cap)),
             "wqa": np.ascontiguousarray(wqa), "wmask": wmask}
        )
    return cap, idxs, counts, in_maps


def run_spmd(query, projected_memory, mask, Wq, We, **spmd_kwargs):
    """Run the compiled kernel on all 8 cores; returns (cap, idxs, counts, res)."""
    cap, idxs, counts, in_maps = _prep(query, projected_memory, mask, Wq, We)
    nc = _get_nc(cap)
    res = run_bass_kernel_spmd(nc, in_maps, list(range(N_CORES)), **spmd_kwargs)
    return cap, idxs, counts, res


def kernel(query, projected_memory, mask, Wq, We):
    cap, idxs, counts, res = run_spmd(query, projected_memory, mask, Wq, We)
    out = np.zeros((B, 1, S), dtype=np.float32)
    for i in range(N_CORES):
        attn = res.results[i]["attn"]
        for bl in range(BL):
            g = i * BL + bl
            cnt = int(counts[g])
            if cnt:
                out[g, 0, idxs[g]] = attn[bl, :cnt].astype(np.float32)
            else:
                out[g, 0, :] = 1.0 / S
    return out
